# revision 16
# baseline (speedup 1.0000x reference)
"""ContextQueryAttention Trainium2 kernel (fp8 DoubleRow edition).

Reference computation (per batch b):
    S = (c@wc)[:,None] + (q@wq)[None,:] + (c*wm) @ q.T        # (Lc, Lq)
    S1 = softmax(S, axis=0)  (over context dim i)
    S2 = softmax(S, axis=1)  (over question dim j)
    A  = S1 @ q
    Bm = (S1 @ S2.T) @ c
    out = [c, A, c*A, c*Bm] @ wr + br

Structure (per batch, building on the algebraic rewrite of the bf16
baseline -- free softmax via q~ = wm*q + wc, Bm = S1 @ (S2.T @ c)):

  * Scores stay float32r: ST = q~T.T @ cT, E1T = exp(ST) stored bf16
    with the s1 row-sums accumulated by the activation.
  * E18 = fp8(E1T * 2^11/s1[j]) -- the column-softmax normalizer is
    folded into the fp8 copy of E1T, so downstream probability-weighted
    matmuls use raw fp8 operands:
      - AT   = q8.T  @ E18            (2^11 A^T)
      - BmT  = Y8.T  @ E18            (2^14 Bm^T)
      - A@W2 term = E18.T @ P28 where P28 = fp8((q@W2) * 2^-2), so the
        (B,Lc,D)-sized A@W2 product never materializes A in fp8.
  * All of those run as fp8e4m3 MatmulPerfMode.DoubleRow (2 k-tiles of
    128 per instruction, 2x PE throughput). Power-of-2 scales keep every
    fp8 tensor inside e4m3 range (max normal 240): G8 = 2^11*G,
    Y8 = 2^3*Y0, AT staging 2^4*A^T, cA/cBm at 2^4, W3/W4 at 2^5,
    and the final PSUM accumulates at Lambda = 2^9 (output scaled 2^-9).
  * The c@W1 term of the output projection stays bf16 (c is O(1) and
    dominates the output; fp8 there would blow the error budget).
  * wr/br are pre-scaled on the host (free) so on-chip loads are pure
    dtype casts.

Sharding: pure data parallel over batch: 16 batches -> 8 cores x 2.
"""

import numpy as np

import concourse.bass as bass
import concourse.tile as tile
from concourse import bacc, mybir
from concourse import bass2jax
from concourse.masks import make_identity

N_CORES = 8
B, Lc, Lq, D = 16, 2048, 512, 512
BPC = B // N_CORES  # batches per core

F32 = mybir.dt.float32
F32R = mybir.dt.float32r
BF = mybir.dt.bfloat16
F8 = mybir.dt.float8e4
DR = mybir.MatmulPerfMode.DoubleRow

AF = mybir.ActivationFunctionType
ALU = mybir.AluOpType
AX = mybir.AxisListType

NT = Lc // 128   # 16 context row-blocks
NG = Lq // 128   # 4 question row-blocks
NK = D // 128    # 4 feature blocks

# fp8 scale exponents (validated against reference data in numpy):
#   E18 = E1T * 2^SE / s1, G8 = 2^SG * G, Y8 = 2^(SG+SY-8)=2^3 * Y0,
#   P28 = (q@W2) * 2^SP, staging AT/Bm at 2^4, W3/W4 at 2^5, Lambda 2^9.
SE = 11    # E18 fold:  2^11/s1
SG = 8     # G8 = 2^8 * S2 (S2 max weight 0.283 -> max 72)
SP = -2    # P28 scale (with E18's 2^11 -> term lands at Lambda=2^9)
SAT = -7   # AT staging: psum(2^11 A^T) * 2^-7 = 2^4 A^T
SBM = -10  # Bm staging: psum(2^14 Bm^T) * 2^-10 = 2^4 Bm^T
SL = 9     # Lambda: final psum scale; output copy multiplies 2^-9


def build_program(hybrid=True, repeat=1):
    nc = bacc.Bacc(None, target_bir_lowering=False)

    # c2/q2 declared float32r: same 4-byte storage as the f32 numpy input,
    # so the fast HWDGE path loads them directly and the PE rounds on read.
    c2 = nc.declare_dram_parameter("c2", [BPC, Lc, D], F32R, isOutput=False)
    q2 = nc.declare_dram_parameter("q2", [BPC, Lq, D], F32R, isOutput=False)
    w0 = nc.declare_dram_parameter("w0", [3 * D], F32, isOutput=False)
    # wrs quarters pre-scaled on host: [W1*2^9, W2, W3*2^5, W4*2^5]
    wrs = nc.declare_dram_parameter("wrs", [4 * D, D], F32, isOutput=False)
    brs = nc.declare_dram_parameter("brs", [D], F32, isOutput=False)  # br*2^9
    out2 = nc.declare_dram_parameter("out2", [BPC, Lc, D], F32, isOutput=True)
    v_dram = nc.dram_tensor("v_scratch", [Lq], F32)

    def load(out, in_):
        # gpsimd (SWDGE) DMAs cast dtype on the fly.
        nc.gpsimd.dma_start(out=out, in_=in_)

    with tile.TileContext(nc) as tc:
        with (
            tc.tile_pool(name="sb", bufs=1) as sb,
            tc.tile_pool(name="ps", bufs=3, space="PSUM") as ps,
            tc.tile_pool(name="pt", bufs=2, space="PSUM") as pt,
        ):
            # ---- constants ----
            ident_f = sb.tile([128, 128], F32, tag="identf")
            make_identity(nc, ident_f)
            ident_r = sb.tile([128, 128], F32R, tag="identr")
            nc.vector.tensor_copy(ident_r, ident_f)
            identb = sb.tile([128, 128], BF, tag="identb")
            nc.vector.tensor_copy(identb, ident_f)
            ones1_f = sb.tile([1, 128], F32, tag="ones1f")
            nc.vector.memset(ones1_f, 1.0)
            ones1b = sb.tile([1, 128], BF, tag="ones1b")
            nc.vector.tensor_copy(ones1b, ones1_f)
            ones8c = sb.tile([128, 1], BF, tag="ones8c")
            nc.vector.memset(ones8c, 2.0 ** (-SG))

            wc_sb = sb.tile([128, NK], F32, tag="wc")
            wm_sb = sb.tile([128, NK], F32, tag="wm")
            wq_sb = sb.tile([128, NK], F32R, tag="wq")
            nc.sync.dma_start(out=wc_sb, in_=w0[0:D].rearrange("(k p) -> p k", p=128))
            nc.sync.dma_start(out=wm_sb, in_=w0[2 * D:3 * D].rearrange("(k p) -> p k", p=128))
            load(wq_sb, w0[D:2 * D].rearrange("(k p) -> p k", p=128))

            brs_sb = sb.tile([1, D], BF, tag="brs")
            load(brs_sb, brs.rearrange("(a e) -> a e", a=1))

            wr_r = wrs.rearrange("(t p) e -> p t e", p=128)
            W1b = sb.tile([128, NK, D], BF, tag="w1b")
            load(W1b, wr_r[:, 0:NK, :])
            W2r = sb.tile([128, NK, D], F32R, tag="w2r")
            load(W2r, wr_r[:, NK:2 * NK, :])
            W38 = sb.tile([128, NK, D], F8, tag="w38")
            load(W38, wr_r[:, 2 * NK:3 * NK, :])
            W48 = sb.tile([128, NK, D], F8, tag="w48")
            load(W48, wr_r[:, 3 * NK:4 * NK, :])

            def one_batch(b):
                # ---- load (HWDGE, f32r == f32 bits) ----
                qSt = sb.tile([128, NG, D], F32R, tag="qst")
                nc.sync.dma_start(out=qSt, in_=q2[b].rearrange("(g p) d -> p g d", p=128))
                cSt = sb.tile([128, NT, D], F32R, tag="cst")
                c_r = c2[b].rearrange("(t p) d -> p t d", p=128)
                for tq in range(4):
                    eng = nc.sync if tq % 2 == 0 else nc.gpsimd
                    eng.dma_start(out=cSt[:, tq * 4:(tq + 1) * 4, :],
                                  in_=c_r[:, tq * 4:(tq + 1) * 4, :])

                # ---- fp8 casts of raw inputs (SWDGE sbuf->sbuf) ----
                cN8 = sb.tile([128, NT, D], F8, tag="cn8")
                for tq in range(4):
                    load(cN8[:, tq * 4:(tq + 1) * 4, :], cSt[:, tq * 4:(tq + 1) * 4, :])
                q8t = sb.tile([128, NG, D], F8, tag="q8")
                load(q8t, qSt)

                # ---- q transposes (f32r, 1.5 cy/row) ----
                qT = sb.tile([128, NK, Lq], F32R, tag="qt")
                for kd in range(NK):
                    ptile = pt.tile([128, 512], F32R, tag="tr")
                    for g in range(NG):
                        nc.tensor.transpose(
                            ptile[:, g * 128:(g + 1) * 128],
                            qSt[:, g, kd * 128:(kd + 1) * 128], ident_r)
                    nc.any.tensor_copy(qT[:, kd, :], ptile)

                # ---- v = q @ wq (row form), then PE-transpose to column ----
                pv = ps.tile([128, 1024], F32, tag="mw")
                for kd in range(NK):
                    nc.tensor.matmul(pv[0:1, 0:512], wq_sb[:, kd:kd + 1], qT[:, kd, :],
                                     start=(kd == 0), stop=(kd == NK - 1))
                v_sb = sb.tile([1, Lq], F32, tag="vrow")
                nc.any.tensor_copy(v_sb, pv[0:1, 0:512])
                nc.sync.dma_start(out=v_dram[:], in_=v_sb[0:1, :])
                vcol = sb.tile([128, NG], F32, tag="vcol")
                nc.sync.dma_start(out=vcol, in_=v_dram.rearrange("(g p) -> p g", p=128))

                # ---- P2 = q @ W2 (f32r), P28 = fp8(psum * 2^SP) ----
                P28 = sb.tile([128, NG, D], F8, tag="p28")
                for g2 in range(2):
                    pm = ps.tile([128, 1024], F32, tag="mw")
                    for half in range(2):
                        g = g2 * 2 + half
                        for kd in range(NK):
                            nc.tensor.matmul(
                                pm[:, half * 512:(half + 1) * 512],
                                qT[:, kd, g * 128:(g + 1) * 128], W2r[:, kd, :],
                                start=(kd == 0), stop=(kd == NK - 1))
                    for half in range(2):
                        g = g2 * 2 + half
                        nc.any.tensor_scalar_mul(
                            P28[:, g, :], pm[:, half * 512:(half + 1) * 512], 2.0 ** SP)

                # ---- q~T = wm * qT + wc (in place, after P2 consumed qT) ----
                for kd in range(NK):
                    nc.vector.tensor_scalar(
                        out=qT[:, kd, :], in0=qT[:, kd, :],
                        scalar1=wm_sb[:, kd:kd + 1], scalar2=wc_sb[:, kd:kd + 1],
                        op0=ALU.mult, op1=ALU.add)

                # ---- c transposes (f32r) -> cT + cTb ----
                cT = sb.tile([128, NK, Lc], F32R, tag="ct")
                cTb = sb.tile([128, NK, Lc], BF, tag="ctb")
                for kd in range(NK):
                    for ic4 in range(4):
                        ptile = pt.tile([128, 512], F32R, tag="tr")
                        for t4 in range(4):
                            t = ic4 * 4 + t4
                            nc.tensor.transpose(
                                ptile[:, t4 * 128:(t4 + 1) * 128],
                                cSt[:, t, kd * 128:(kd + 1) * 128], ident_r)
                        sl = slice(ic4 * 512, (ic4 + 1) * 512)
                        nc.any.tensor_copy(cT[:, kd, sl], ptile)
                    load(cTb[:, kd, :], cT[:, kd, :])

                # ---- ST = q~T.T @ cT -> E1T = exp(ST) bf16; s1; E18 fp8 ----
                E1T = sb.tile([128, NG, Lc], BF, tag="e1t")
                E18 = sb.tile([128, NG, Lc], F8, tag="e18")
                s1p = sb.tile([128, NG, 2], F32, tag="s1p")
                s1s = sb.tile([128, NG], F32, tag="s1s")
                i1s = sb.tile([128, NG], F32, tag="i1s")
                for g in range(NG):
                    for ic2 in range(2):
                        pm = ps.tile([128, 1024], F32, tag="mw")
                        for half in range(2):
                            ic = ic2 * 2 + half
                            for kd in range(NK):
                                nc.tensor.matmul(
                                    pm[:, half * 512:(half + 1) * 512],
                                    qT[:, kd, g * 128:(g + 1) * 128],
                                    cT[:, kd, ic * 512:(ic + 1) * 512],
                                    start=(kd == 0), stop=(kd == NK - 1))
                        nc.scalar.activation(
                            out=E1T[:, g, ic2 * 1024:(ic2 + 1) * 1024], in_=pm,
                            func=AF.Exp, bias=vcol[:, g:g + 1],
                            accum_out=s1p[:, g, ic2:ic2 + 1])
                    nc.vector.reduce_sum(out=s1s[:, g:g + 1], in_=s1p[:, g, :], axis=AX.X)
                    nc.vector.reciprocal(out=i1s[:, g:g + 1], in_=s1s[:, g:g + 1])
                    nc.vector.tensor_scalar_mul(i1s[:, g:g + 1], i1s[:, g:g + 1], 2.0 ** SE)
                    for ic2 in range(2):
                        sl = slice(ic2 * 1024, (ic2 + 1) * 1024)
                        nc.any.tensor_scalar_mul(E18[:, g, sl], E1T[:, g, sl], i1s[:, g:g + 1])

                # ---- s2 column form: s2[i] = sum_j E1T[j,i] * 2^-SG
                #      64 single-col matmuls, zero-first bank + accumulate ----
                ps2 = ps.tile([128, 1024], F32, tag="mw")
                for t in range(NT):
                    for g in range(NG):
                        first = (t == 0 and g == 0)
                        last = (t == NT - 1 and g == NG - 1)
                        nc.tensor.matmul(
                            ps2[:, t:t + 1], E1T[:, g, t * 128:(t + 1) * 128],
                            ones8c, start=first, stop=last,
                            skip_group_check=True)
                invs2c = sb.tile([128, NT], F32, tag="invs2c")  # = 2^SG / s2
                nc.vector.reciprocal(out=invs2c, in_=ps2[:, 0:NT])

                # ---- G8[i,j] = fp8(E1T[j,i] * 2^SG/s2[i]) via bf16 transposes ----
                G8 = sb.tile([128, NT, Lq], F8, tag="g8")
                for t in range(NT):
                    ptb = pt.tile([128, 512], BF, tag="tr")
                    for g in range(NG):
                        nc.tensor.transpose(
                            ptb[:, g * 128:(g + 1) * 128],
                            E1T[:, g, t * 128:(t + 1) * 128], identb)
                    nc.any.tensor_scalar_mul(G8[:, t, :], ptb, invs2c[:, t:t + 1])

                # ---- Y8 = fp8((G8.T @ cN8) * ev*2^SYC)  [fp8 DoubleRow] ----
                Y8 = sb.tile([128, NG, D], F8, tag="y8")
                for g2 in range(2):
                    pm = ps.tile([128, 1024], F32, tag="mw")
                    for half in range(2):
                        g = g2 * 2 + half
                        for tp in range(NT // 2):
                            nc.tensor.matmul(
                                pm[:, half * 512:(half + 1) * 512],
                                G8[:, 2 * tp:2 * tp + 2, g * 128:(g + 1) * 128],
                                cN8[:, 2 * tp:2 * tp + 2, :],
                                start=(tp == 0), stop=(tp == NT // 2 - 1),
                                perf_mode=DR)
                    for half in range(2):
                        g = g2 * 2 + half
                        nc.any.tensor_scalar_mul(
                            Y8[:, g, :], pm[:, half * 512:(half + 1) * 512],
                            2.0 ** (3 - SG))

                # ---- AT = q8.T @ E18 (2^SE A^T); cAT8 = (psum*2^-7)*cTb ----
                cAT8 = sb.tile([128, NK, Lc], F8, tag="cat8")
                for kd in range(NK):
                    for ic2 in range(2):
                        pm = ps.tile([128, 1024], F32, tag="mw")
                        for half in range(2):
                            ic = ic2 * 2 + half
                            for gp in range(NG // 2):
                                nc.tensor.matmul(
                                    pm[:, half * 512:(half + 1) * 512],
                                    q8t[:, 2 * gp:2 * gp + 2, kd * 128:(kd + 1) * 128],
                                    E18[:, 2 * gp:2 * gp + 2, ic * 512:(ic + 1) * 512],
                                    start=(gp == 0), stop=(gp == NG // 2 - 1),
                                    perf_mode=DR)
                        for half in range(2):
                            sl = slice(ic2 * 1024 + half * 512,
                                       ic2 * 1024 + (half + 1) * 512)
                            nc.vector.scalar_tensor_tensor(
                                out=cAT8[:, kd, sl],
                                in0=pm[:, half * 512:(half + 1) * 512],
                                scalar=2.0 ** SAT, in1=cTb[:, kd, sl],
                                op0=ALU.mult, op1=ALU.mult)

                # ---- BmT = Y8.T @ E18 (2^14 Bm^T); cBmT8 = (psum*2^-10)*cTb ----
                cBmT8 = sb.tile([128, NK, Lc], F8, tag="cbt8")
                for kd in range(NK):
                    for ic2 in range(2):
                        pm = ps.tile([128, 1024], F32, tag="mw")
                        for half in range(2):
                            ic = ic2 * 2 + half
                            for gp in range(NG // 2):
                                nc.tensor.matmul(
                                    pm[:, half * 512:(half + 1) * 512],
                                    Y8[:, 2 * gp:2 * gp + 2, kd * 128:(kd + 1) * 128],
                                    E18[:, 2 * gp:2 * gp + 2, ic * 512:(ic + 1) * 512],
                                    start=(gp == 0), stop=(gp == NG // 2 - 1),
                                    perf_mode=DR)
                        for half in range(2):
                            sl = slice(ic2 * 1024 + half * 512,
                                       ic2 * 1024 + (half + 1) * 512)
                            nc.vector.scalar_tensor_tensor(
                                out=cBmT8[:, kd, sl],
                                in0=pm[:, half * 512:(half + 1) * 512],
                                scalar=2.0 ** SBM, in1=cTb[:, kd, sl],
                                op0=ALU.mult, op1=ALU.mult)

                # ---- out*2^SL = c@W1b + E18.T@P28 + cAT8.T@W38 + cBmT8.T@W48 + br*2^SL ----
                for t2 in range(NT // 2):
                    pm = ps.tile([128, 1024], F32, tag="mw")
                    for half in range(2):
                        t = t2 * 2 + half
                        o = pm[:, half * 512:(half + 1) * 512]
                        for kd in range(NK):
                            nc.tensor.matmul(o, cTb[:, kd, t * 128:(t + 1) * 128],
                                             W1b[:, kd, :], start=(kd == 0), stop=False)
                        for gp in range(NG // 2):
                            nc.tensor.matmul(
                                o, E18[:, 2 * gp:2 * gp + 2, t * 128:(t + 1) * 128],
                                P28[:, 2 * gp:2 * gp + 2, :],
                                start=False, stop=False, perf_mode=DR)
                        for kp in range(NK // 2):
                            nc.tensor.matmul(
                                o, cAT8[:, 2 * kp:2 * kp + 2, t * 128:(t + 1) * 128],
                                W38[:, 2 * kp:2 * kp + 2, :],
                                start=False, stop=False, perf_mode=DR)
                        for kp in range(NK // 2):
                            nc.tensor.matmul(
                                o, cBmT8[:, 2 * kp:2 * kp + 2, t * 128:(t + 1) * 128],
                                W48[:, 2 * kp:2 * kp + 2, :],
                                start=False, stop=False, perf_mode=DR)
                        nc.tensor.matmul(o, ones1b, brs_sb, start=False, stop=True)
                    ot = sb.tile([128, 2, 512], F32, tag="outst", bufs=3)
                    nc.any.tensor_scalar_mul(ot, pm, 2.0 ** (-SL))
                    nc.sync.dma_start(
                        out=out2[b].rearrange("(u p) e -> p u e", p=128)[:, t2 * 2:t2 * 2 + 2, :],
                        in_=ot)

            if repeat > 1:
                # timing harness only: repeat the whole workload on-device so
                # per-call dispatch overhead can be subtracted out
                hints = (mybir.EngineType.PE, mybir.EngineType.DVE,
                         mybir.EngineType.Activation, mybir.EngineType.SP,
                         mybir.EngineType.Pool)
                with tc.For_i(0, repeat, 1, hint_engines=hints):
                    for b in range(BPC):
                        one_batch(b)
            else:
                for b in range(BPC):
                    one_batch(b)

    nc.compile()
    return nc


class Runner:
    """Persistent SPMD runner: jit once, execute many times.

    Mirrors concourse.bass2jax.run_bass_via_pjrt's multi-core path but keeps
    the compiled executable so repeated calls don't recompile.
    """

    def __init__(self, nc):
        import jax
        from jax.experimental.shard_map import shard_map
        from jax.sharding import Mesh, PartitionSpec

        bass2jax.install_neuronx_cc_hook()
        self.nc = nc
        self.jax = jax

        partition_name = (
            nc.partition_id_tensor.name if nc.partition_id_tensor else None
        )
        in_names, out_names, out_avals, zero_shapes = [], [], [], []
        for alloc in nc.m.functions[0].allocations:
            if not isinstance(alloc, mybir.MemoryLocationSet):
                continue
            name = alloc.memorylocations[0].name
            if alloc.kind == "ExternalInput":
                if name != partition_name:
                    in_names.append(name)
            elif alloc.kind == "ExternalOutput":
                shape = tuple(alloc.tensor_shape)
                dtype = mybir.dt.np(alloc.dtype)
                out_names.append(name)
                out_avals.append(jax.core.ShapedArray(shape, dtype))
                zero_shapes.append((shape, dtype))
        self.in_names = list(in_names)
        self.out_names = out_names
        self.out_avals = out_avals
        self.zero_shapes = zero_shapes
        n_params = len(in_names)
        n_outs = len(out_names)

        all_in_names = list(in_names) + list(out_names)
        if partition_name is not None:
            all_in_names.append(partition_name)

        def _body(*args):
            operands = list(args)
            if partition_name is not None:
                operands.append(bass2jax.partition_id_tensor())
            outs = bass2jax._bass_exec_p.bind(
                *operands,
                out_avals=tuple(out_avals),
                in_names=tuple(all_in_names),
                out_names=tuple(out_names),
                lowering_input_output_aliases=(),
                sim_require_finite=True,
                sim_require_nnan=True,
                nc=nc,
            )
            return tuple(outs)

        devices = jax.devices()[:N_CORES]
        mesh = Mesh(np.asarray(devices), ("core",))
        in_specs = (PartitionSpec("core"),) * (n_params + n_outs)
        out_specs = (PartitionSpec("core"),) * n_outs
        self.fn = jax.jit(
            shard_map(_body, mesh=mesh, in_specs=in_specs,
                      out_specs=out_specs, check_rep=False),
            keep_unused=True,
        )

    def concat_inputs(self, in_maps):
        return [
            np.concatenate([np.asarray(m[name]) for m in in_maps], axis=0)
            for name in self.in_names
        ]

    def zeros(self):
        return [
            np.zeros((N_CORES * s[0], *s[1:]), d) for (s, d) in self.zero_shapes
        ]

    def run_device(self, concat_in, zeros):
        """Execute; returns list of global (N_CORES*dim0, ...) jax arrays."""
        out = self.fn(*concat_in, *zeros)
        self.jax.block_until_ready(out)
        return out

    def run(self, in_maps):
        outs = self.run_device(self.concat_inputs(in_maps), self.zeros())
        return [
            {
                name: np.asarray(outs[i]).reshape(
                    N_CORES, *self.out_avals[i].shape)[c]
                for i, name in enumerate(self.out_names)
            }
            for c in range(N_CORES)
        ]


_CACHED = {}


def _get_runner(**kw):
    key = tuple(sorted(kw.items()))
    if key not in _CACHED:
        _CACHED[key] = Runner(build_program(**kw))
    return _CACHED[key]


def make_in_maps(context, question, w0, wr, br):
    wr = np.asarray(wr, dtype=np.float32)
    wrs = np.concatenate([
        wr[0:D] * 2.0 ** SL,
        wr[D:2 * D],
        wr[2 * D:3 * D] * 2.0 ** 5,
        wr[3 * D:4 * D] * 2.0 ** 5,
    ], axis=0)
    brs = np.asarray(br, dtype=np.float32) * 2.0 ** SL
    return [
        {
            "c2": context[c * BPC:(c + 1) * BPC],
            "q2": question[c * BPC:(c + 1) * BPC],
            "w0": w0,
            "wrs": wrs,
            "brs": brs,
        }
        for c in range(N_CORES)
    ]


def kernel(context, question, w0, wr, br):
    context = np.ascontiguousarray(np.asarray(context, dtype=np.float32))
    question = np.ascontiguousarray(np.asarray(question, dtype=np.float32))
    w0 = np.ascontiguousarray(np.asarray(w0, dtype=np.float32))
    wr = np.ascontiguousarray(np.asarray(wr, dtype=np.float32))
    br = np.ascontiguousarray(np.asarray(br, dtype=np.float32))

    runner = _get_runner()
    res = runner.run(make_in_maps(context, question, w0, wr, br))
    return np.concatenate([res[c]["out2"] for c in range(N_CORES)], axis=0)
